# revision 27
# baseline (speedup 1.0000x reference)
# MultiLoraConv2d kernel for 8 trn2 NeuronCores (Bass/Tile, data-parallel over batch).
#
# Math (per sample b):
#   delta_flat[b] = sum_t 2*alphas[b,t] * (lora_B[t] @ lora_A[t])        [768, 768]
#   agg[b] = W + delta_flat[b].reshape(COUT, CIN, 3, 3)                  (flat reinterpret)
#   out[b] = conv2d(x[b], agg[b], pad=1)
#
# Device strategy (per core, S = B/8 samples):
#   - All matmul operands bf16 (1 cycle/row on PE, same as fp32r; halves DMA
#     + SBUF). PSUM accumulation stays fp32; max rel err ~2.3e-3 (gate 2e-2).
#   - Host pre-lays-out tensors partition-first; LoRA factors regrouped by
#     (d = 3*i + j, s = (c*9+d)//768) so per-sample aggregated conv weights
#     come out of the PE directly in c-major (stationary) layout:
#       S_d[c, o] = sum_s sum_r a3[d,s,r,c] * (2*alpha_{t(r)} * b3[s,r,o])
#   - Conv = 18 shifted matmuls (9 taps x 2 cin tiles) per PSUM bank;
#     measured issue cadence ~218 ns per 512-row bf16 matmul (hw floor).
#   - Schedule: 11 dummy warmup matmuls ramp the PE p-state during the ~8us
#     NEFF boot + ~11us first-DMA latency window; DMAs issued in consumption
#     order (alph/b3[s0] first, a3/wt interleaved per-d, then x prefetches);
#     b3s muls split DVE/Act in consumption order; phase-1 evictions on DVE
#     (GpSimd cannot touch PSUM and is ~10x slower on elementwise); conv
#     PSUM->SBUF copies alternate DVE/Act; single 8-bank PSUM pool (6 dp +
#     2 pb ring slots, warmup folded into the dp tag); output batched 1 DMA
#     per (smp, ot), final group split fine to shrink the drain tail.
import numpy as np

B, T, R, ALPHA = 32, 4, 8, 16
CIN, COUT, K = 256, 256, 3
H = W_SP = 64
SCALING = ALPHA / R
NCORES = 8
S = B // NCORES      # samples per core
NR = T * R * K       # 96 lora rows (padded to 128 partitions)
P = 128
HP = H + 2           # 66 padded

_CACHE = {}


def _build_nc():
    import concourse.bacc as bacc
    import concourse.mybir as mybir
    import concourse.tile as tile

    f32 = mybir.dt.float32
    bf16 = mybir.dt.bfloat16

    nc = bacc.Bacc("TRN2", target_bir_lowering=False, debug=False, num_devices=NCORES)

    xp = nc.declare_dram_parameter("xp", [S, 2, P, HP, HP], bf16, isOutput=False)
    wt = nc.declare_dram_parameter("wt", [P, 9, 2, 2, COUT], bf16, isOutput=False)
    a3 = nc.declare_dram_parameter("a3", [P, 9, 3, CIN], bf16, isOutput=False)
    b3 = nc.declare_dram_parameter("b3", [P, 3, COUT], bf16, isOutput=False)
    alph = nc.declare_dram_parameter("alph", [P, S], f32, isOutput=False)
    outd = nc.declare_dram_parameter("out", [S, 2, P, H, W_SP], f32, isOutput=True)

    with tile.TileContext(nc) as tc:
        with tc.tile_pool(name="persist", bufs=1) as persist, \
             tc.tile_pool(name="xt_pool", bufs=6) as xt_pool, \
             tc.tile_pool(name="ob_pool", bufs=2) as ob_pool, \
             tc.tile_pool(name="ps", bufs=1, space="PSUM") as ps:
            a3_sb = persist.tile([P, 9, 3, CIN], bf16)
            b3_sb = persist.tile([P, 3, COUT], bf16)
            alph_sb = persist.tile([P, S], f32)
            wt_sb = persist.tile([P, 9, 2, 2, COUT], bf16)
            b3s = [persist.tile([P, 3, 2, COUT], bf16, name=f"b3s{h}")
                   for h in range(2)]
            ws2 = [persist.tile([P, 9, S, COUT], bf16, name=f"ws2{c}")
                   for c in range(2)]
            wz = persist.tile([P, 704], bf16)

            # PE warmup: p-state ramps to full clock after ~3us of continuous
            # execution; burn that in during NEFF boot / first DMAs so real
            # matmuls run near 2.4GHz from the start. ~11x512-row at mid clock
            # ends ~11.1us, right when the first dp group's operands land
            # (alph DMA completes ~10.9 + two muls). memset on DVE (GpSimd's
            # boot is slow/variable); a dummy Act copy pulls the 1.3us
            # ACT_TABLE_LOAD off the b3s-mul critical path (its target range
            # is outside the warmup APs so it gates nothing).
            nc.vector.memset(wz[:, :], 0.0)
            nc.scalar.copy(wz[:, 672:688], wz[:, 640:656])
            warm = ps.tile([P, 2, COUT], f32, name="dp", bufs=6)
            for _ in range(9):
                nc.tensor.matmul(warm[:, :, :], wz[:, 0:128], wz[:, 128:640],
                                 start=True, stop=True)

            # DMAs in consumption order, split across BOTH hardware DGE
            # queues (SP + Activation) because the early-window per-queue
            # rate (~0.19 MB/us) can't feed the phase-1 d-loop alone:
            # a3 (stationary, gates matmuls) on SP; wt (gates only the
            # ring-buffered eviction adds) on Act. alph (first transfer,
            # ~10.9us completion due to DMA pipe latency) gates the b3s muls;
            # x prefetches trail (needed only when conv starts ~39us).
            nc.sync.dma_start(alph_sb[:, :], alph[:, :])
            nc.sync.dma_start(b3_sb[:, 0:1], b3[:, 0:1])
            nc.sync.dma_start(b3_sb[:, 1:3], b3[:, 1:3])
            nc.sync.dma_start(a3_sb[:, 0:1], a3[:, 0:1])
            nc.sync.dma_start(a3_sb[:, 1:3], a3[:, 1:3])
            nc.sync.dma_start(a3_sb[:, 3:6], a3[:, 3:6])
            nc.sync.dma_start(a3_sb[:, 6:9], a3[:, 6:9])
            nc.scalar.dma_start(wt_sb[:, 0:1], wt[:, 0:1])
            nc.scalar.dma_start(wt_sb[:, 1:3], wt[:, 1:3])
            nc.scalar.dma_start(wt_sb[:, 3:6], wt[:, 3:6])
            nc.scalar.dma_start(wt_sb[:, 6:9], wt[:, 6:9])
            # (wt is host-doubled along a j=2 dim so phase-1 eviction is a
            # single [P,2,256] tensor_add per (h,d,ct) with no broadcast AP)
            xts = [[None, None] for _ in range(S)]
            for smp in range(3):
                for ct in range(2):
                    t = xt_pool.tile([P, HP, HP], bf16, name="xt")
                    nc.sync.dma_start(t[:, :, :], xp[smp, ct, :, :, :])
                    xts[smp][ct] = t

            # b3s[h][:, s, j, :] = (2*alphas[2h+j]) * b3[s]  (bf16 out; the
            # 2x scaling is folded into the host-side alph values).
            # DVE and Act interleave so the first dp group's s=0,1,2 operands
            # land in consumption order with no single-engine serial chain;
            # DVE then goes straight into the h=0 eviction adds.
            for s, j, eng in ((0, 0, nc.vector), (0, 1, nc.vector),
                              (2, 0, nc.vector), (2, 1, nc.vector)):
                eng.tensor_scalar_mul(b3s[0][:, s, j, :], b3_sb[:, s, :],
                                      alph_sb[:, j:j + 1])
            for h, s, j in ((0, 1, 0), (0, 1, 1), (1, 0, 0), (1, 0, 1),
                            (1, 1, 0), (1, 1, 1), (1, 2, 0), (1, 2, 1)):
                nc.scalar.mul(b3s[h][:, s, j, :], b3_sb[:, s, :],
                              alph_sb[:, 2 * h + j:2 * h + j + 1])

            # ---- phase 1: aggregated weights via LoRA matmuls ----
            # dp[c, j, o] = sum_s sum_r a3[d,s,r,c] * b3s[h][s,j,o]; then
            # ws2[ct][c, d, 2h+j, o] = dp[c, j, o] + W[c, d, ct, o].
            for h in range(2):
                for d in range(9):
                    for ct in range(2):
                        dp = ps.tile([P, 2, COUT], f32, name="dp", bufs=6)
                        for s in range(3):
                            nc.tensor.matmul(
                                dp[:, :, :],
                                a3_sb[:, d, s, ct * P:(ct + 1) * P],
                                b3s[h][:, s, :, :],
                                start=(s == 0), stop=(s == 2))
                        nc.vector.tensor_add(
                            ws2[ct][:, d, 2 * h:2 * h + 2, :],
                            dp[:, :, :], wt_sb[:, d, ct, :, :])

            # ---- phase 2: per-sample conv, 18 shifted matmuls per psum bank ----
            for smp in range(S):
                if smp == 1:
                    for ct in range(2):
                        t = xt_pool.tile([P, HP, HP], bf16, name="xt")
                        nc.sync.dma_start(t[:, :, :], xp[3, ct, :, :, :])
                        xts[3][ct] = t
                for ot in range(2):
                    ob = ob_pool.tile([P, H, W_SP], f32, name="ob")
                    last = (smp == S - 1)
                    for hb in range(8):
                        pb = ps.tile([P, 8, W_SP], f32, name="pb", bufs=2)
                        first = True
                        for ct in range(2):
                            for d in range(9):
                                di, dj = divmod(d, 3)
                                loc = hb * 8 + di
                                nc.tensor.matmul(
                                    pb[:, :, :],
                                    ws2[ct][:, d, smp, ot * P:(ot + 1) * P],
                                    xts[smp][ct][:, loc:loc + 8, dj:dj + W_SP],
                                    start=first, stop=(ct == 1 and d == 8))
                                first = False
                        if hb % 2 == 0:
                            nc.vector.tensor_copy(
                                ob[:, hb * 8:(hb + 1) * 8, :], pb[:, :, :])
                        else:
                            nc.scalar.copy(
                                ob[:, hb * 8:(hb + 1) * 8, :], pb[:, :, :])
                        if last and ot == 1 and hb >= 4:
                            nc.sync.dma_start(
                                outd[smp, ot, :, hb * 8:(hb + 1) * 8, :],
                                ob[:, hb * 8:(hb + 1) * 8, :])
                        elif last and hb % 2 == 1:
                            k = hb // 2
                            nc.sync.dma_start(
                                outd[smp, ot, :, k * 16:(k + 1) * 16, :],
                                ob[:, k * 16:(k + 1) * 16, :])
                    if not last:
                        nc.sync.dma_start(outd[smp, ot, :, :, :], ob[:, :, :])
    nc.finalize()
    return nc


def _host_prep(x, alphas, W, lora_A, lora_B):
    """Host-side layout/dtype transforms (pad/transpose/gather/cast)."""
    import ml_dtypes
    bf16 = ml_dtypes.bfloat16

    xf = np.asarray(x, dtype=np.float32)
    af = np.asarray(alphas, dtype=np.float32)
    Wf = np.asarray(W, dtype=np.float32)
    Af = np.asarray(lora_A, dtype=np.float32).reshape(NR, CIN * K)   # Acat
    Bf = np.asarray(lora_B, dtype=np.float32)

    # padded x, per core: (S, 2, 128, 66, 66) bf16
    xpad = np.zeros((B, CIN, HP, HP), bf16)
    xpad[:, :, 1:-1, 1:-1] = xf.astype(bf16)
    xpad = xpad.reshape(NCORES, S, 2, P, HP, HP)

    # base weights c-major, d-major free layout, doubled along j so the
    # device-side eviction add needs no broadcast: wt[p, d, ct, j, o]
    wth = np.ascontiguousarray(
        Wf.reshape(COUT, CIN, 9).transpose(1, 2, 0)        # [c, d, o]
        .reshape(2, P, 9, COUT)                            # [ct, p, d, o]
        .transpose(1, 2, 0, 3)).astype(bf16)               # [p, d, ct, o]
    wth = np.ascontiguousarray(
        np.broadcast_to(wth[:, :, :, None, :], (P, 9, 2, 2, COUT)))

    # a3[r, d, s, c] = Acat[r, c*9+d-768*s] masked; rows padded 96 -> 128
    a3h = np.zeros((P, 9, 3, CIN), np.float32)
    cc = np.arange(CIN)
    for d in range(9):
        q = cc * 9 + d
        s_of_c = q // (CIN * K)
        q_of_c = q % (CIN * K)
        for s in range(3):
            m = s_of_c == s
            a3h[:NR, d, s, m] = Af[:, q_of_c[m]]
    a3h = a3h.astype(bf16)

    # b3[r, s, o] = Bcat[3o+s, r];  Bcat = lora_B transposed to [768, 96]
    Bcat = Bf.transpose(1, 0, 2).reshape(COUT * K, NR)
    b3h = np.zeros((P, 3, COUT), np.float32)
    b3h[:NR] = Bcat.reshape(COUT, 3, NR).transpose(2, 1, 0)
    b3h = b3h.astype(bf16)

    # alph[r, smp] per core (repeat each task 24x; zero rows >= 96).
    # SCALING (alpha/r = 2) folded in here so the device skips one mul.
    alphh = np.zeros((NCORES, P, S), np.float32)
    rep = np.repeat(af * SCALING, R * K, axis=1)           # [B, 96]
    alphh[:, :NR, :] = rep.reshape(NCORES, S, NR).transpose(0, 2, 1)

    return xpad, wth, a3h, b3h, alphh


def kernel(x, alphas, W, lora_A, lora_B):
    from concourse.bass_utils import run_bass_kernel_spmd

    if "nc" not in _CACHE:
        _CACHE["nc"] = _build_nc()
    nc = _CACHE["nc"]

    xpad, wth, a3h, b3h, alphh = _host_prep(x, alphas, W, lora_A, lora_B)
    in_maps = [
        {"xp": np.ascontiguousarray(xpad[c]), "wt": wth, "a3": a3h, "b3": b3h,
         "alph": np.ascontiguousarray(alphh[c])}
        for c in range(NCORES)
    ]
    res = run_bass_kernel_spmd(nc, in_maps, list(range(NCORES)))
    out = np.empty((B, COUT, H, W_SP), np.float32)
    for c in range(NCORES):
        out[c * S:(c + 1) * S] = res.results[c]["out"].reshape(S, COUT, H, W_SP)
    return out


# revision 28
# speedup vs baseline: 1.2037x; 1.2037x over previous
# MultiLoraConv2d kernel for 8 trn2 NeuronCores (Bass/Tile, data-parallel over batch).
#
# Math (per sample b):
#   delta_flat[b] = sum_t 2*alphas[b,t] * (lora_B[t] @ lora_A[t])        [768, 768]
#   agg[b] = W + delta_flat[b].reshape(COUT, CIN, 3, 3)                  (flat reinterpret)
#   out[b] = conv2d(x[b], agg[b], pad=1)
#
# Device strategy (per core, S = B/8 samples):
#   - All matmul operands bf16 (1 cycle/row on PE, same as fp32r; halves DMA
#     + SBUF). PSUM accumulation stays fp32; max rel err ~2.3e-3 (gate 2e-2).
#   - Host pre-lays-out tensors partition-first; LoRA factors regrouped by
#     (d = 3*i + j, s = (c*9+d)//768) so per-sample aggregated conv weights
#     come out of the PE directly in c-major (stationary) layout:
#       S_d[c, o] = sum_s sum_r a3[d,s,r,c] * (2*alpha_{t(r)} * b3[s,r,o])
#   - Conv = 18 shifted matmuls (9 taps x 2 cin tiles) per PSUM bank;
#     measured issue cadence ~218 ns per 512-row bf16 matmul (hw floor).
#   - Schedule: 11 dummy warmup matmuls ramp the PE p-state during the ~8us
#     NEFF boot + ~11us first-DMA latency window; DMAs issued in consumption
#     order (alph/b3[s0] first, a3/wt interleaved per-d, then x prefetches);
#     b3s muls split DVE/Act in consumption order; phase-1 evictions on DVE
#     (GpSimd cannot touch PSUM and is ~10x slower on elementwise); conv
#     PSUM->SBUF copies alternate DVE/Act; single 8-bank PSUM pool (6 dp +
#     2 pb ring slots, warmup folded into the dp tag); output batched 1 DMA
#     per (smp, ot), final group split fine to shrink the drain tail.
import numpy as np

B, T, R, ALPHA = 32, 4, 8, 16
CIN, COUT, K = 256, 256, 3
H = W_SP = 64
SCALING = ALPHA / R
NCORES = 8
S = B // NCORES      # samples per core
NR = T * R * K       # 96 lora rows (padded to 128 partitions)
P = 128
HP = H + 2           # 66 padded

_CACHE = {}


def _build_nc():
    import concourse.bacc as bacc
    import concourse.mybir as mybir
    import concourse.tile as tile

    f32 = mybir.dt.float32
    bf16 = mybir.dt.bfloat16

    nc = bacc.Bacc("TRN2", target_bir_lowering=False, debug=False, num_devices=NCORES)

    xp = nc.declare_dram_parameter("xp", [S, 2, P, HP, HP], bf16, isOutput=False)
    wt = nc.declare_dram_parameter("wt", [P, 9, 2, 2, COUT], bf16, isOutput=False)
    a3 = nc.declare_dram_parameter("a3", [P, 9, 3, CIN], bf16, isOutput=False)
    b3 = nc.declare_dram_parameter("b3", [P, 3, COUT], bf16, isOutput=False)
    alph = nc.declare_dram_parameter("alph", [P, S], f32, isOutput=False)
    outd = nc.declare_dram_parameter("out", [S, 2, P, H, W_SP], f32, isOutput=True)

    with tile.TileContext(nc) as tc:
        with tc.tile_pool(name="persist", bufs=1) as persist, \
             tc.tile_pool(name="xt_pool", bufs=6) as xt_pool, \
             tc.tile_pool(name="ob_pool", bufs=2) as ob_pool, \
             tc.tile_pool(name="ps", bufs=1, space="PSUM") as ps:
            a3_sb = persist.tile([P, 9, 3, CIN], bf16)
            b3_sb = persist.tile([P, 3, COUT], bf16)
            alph_sb = persist.tile([P, S], f32)
            wt_sb = persist.tile([P, 9, 2, 2, COUT], bf16)
            b3s = [persist.tile([P, 3, 2, COUT], bf16, name=f"b3s{h}")
                   for h in range(2)]
            ws2 = [persist.tile([P, 9, S, COUT], bf16, name=f"ws2{c}")
                   for c in range(2)]
            wz = persist.tile([P, 704], bf16)

            # PE warmup: p-state ramps to full clock after ~3us of continuous
            # execution; burn that in during NEFF boot / first DMAs so real
            # matmuls run near 2.4GHz from the start. ~11x512-row at mid clock
            # ends ~11.1us, right when the first dp group's operands land
            # (alph DMA completes ~10.9 + two muls). memset on DVE (GpSimd's
            # boot is slow/variable); a dummy Act copy pulls the 1.3us
            # ACT_TABLE_LOAD off the b3s-mul critical path (its target range
            # is outside the warmup APs so it gates nothing).
            nc.vector.memset(wz[:, :], 0.0)
            nc.scalar.copy(wz[:, 672:688], wz[:, 640:656])
            warm = ps.tile([P, 2, COUT], f32, name="dp", bufs=6)
            for _ in range(9):
                nc.tensor.matmul(warm[:, :, :], wz[:, 0:128], wz[:, 128:640],
                                 start=True, stop=True)

            # DMAs in consumption order on the SP queue (issuing from a
            # second DGE queue halves per-queue DMA bandwidth kernel-wide —
            # measured +60us). a3 chunks (stationary, gate matmuls directly)
            # lead their wt peers (which only gate the ring-buffered eviction
            # adds); alph (first transfer, ~10.9us completion due to DMA pipe
            # latency) gates the b3s muls; x prefetches trail (needed ~39us).
            nc.sync.dma_start(alph_sb[:, :], alph[:, :])
            nc.sync.dma_start(b3_sb[:, 0:1], b3[:, 0:1])
            nc.sync.dma_start(b3_sb[:, 1:3], b3[:, 1:3])
            nc.sync.dma_start(a3_sb[:, 0:1], a3[:, 0:1])
            nc.sync.dma_start(a3_sb[:, 1:3], a3[:, 1:3])
            nc.sync.dma_start(wt_sb[:, 0:1], wt[:, 0:1])
            nc.sync.dma_start(a3_sb[:, 3:6], a3[:, 3:6])
            nc.sync.dma_start(wt_sb[:, 1:3], wt[:, 1:3])
            nc.sync.dma_start(a3_sb[:, 6:9], a3[:, 6:9])
            nc.sync.dma_start(wt_sb[:, 3:6], wt[:, 3:6])
            nc.sync.dma_start(wt_sb[:, 6:9], wt[:, 6:9])
            # (wt is host-doubled along a j=2 dim so phase-1 eviction is a
            # single [P,2,256] tensor_add per (h,d,ct) with no broadcast AP)
            xts = [[None, None] for _ in range(S)]
            for smp in range(3):
                for ct in range(2):
                    t = xt_pool.tile([P, HP, HP], bf16, name="xt")
                    nc.sync.dma_start(t[:, :, :], xp[smp, ct, :, :, :])
                    xts[smp][ct] = t

            # b3s[h][:, s, j, :] = (2*alphas[2h+j]) * b3[s]  (bf16 out; the
            # 2x scaling is folded into the host-side alph values).
            # DVE and Act interleave so the first dp group's s=0,1,2 operands
            # land in consumption order with no single-engine serial chain;
            # DVE then goes straight into the h=0 eviction adds.
            for s, j, eng in ((0, 0, nc.vector), (0, 1, nc.vector),
                              (2, 0, nc.vector), (2, 1, nc.vector)):
                eng.tensor_scalar_mul(b3s[0][:, s, j, :], b3_sb[:, s, :],
                                      alph_sb[:, j:j + 1])
            for h, s, j in ((0, 1, 0), (0, 1, 1), (1, 0, 0), (1, 0, 1),
                            (1, 1, 0), (1, 1, 1), (1, 2, 0), (1, 2, 1)):
                nc.scalar.mul(b3s[h][:, s, j, :], b3_sb[:, s, :],
                              alph_sb[:, 2 * h + j:2 * h + j + 1])

            # ---- phase 1: aggregated weights via LoRA matmuls ----
            # dp[c, j, o] = sum_s sum_r a3[d,s,r,c] * b3s[h][s,j,o]; then
            # ws2[ct][c, d, 2h+j, o] = dp[c, j, o] + W[c, d, ct, o].
            for h in range(2):
                for d in range(9):
                    for ct in range(2):
                        dp = ps.tile([P, 2, COUT], f32, name="dp", bufs=6)
                        for s in range(3):
                            nc.tensor.matmul(
                                dp[:, :, :],
                                a3_sb[:, d, s, ct * P:(ct + 1) * P],
                                b3s[h][:, s, :, :],
                                start=(s == 0), stop=(s == 2))
                        nc.vector.tensor_add(
                            ws2[ct][:, d, 2 * h:2 * h + 2, :],
                            dp[:, :, :], wt_sb[:, d, ct, :, :])

            # ---- phase 2: per-sample conv, 18 shifted matmuls per psum bank ----
            for smp in range(S):
                if smp == 1:
                    for ct in range(2):
                        t = xt_pool.tile([P, HP, HP], bf16, name="xt")
                        nc.sync.dma_start(t[:, :, :], xp[3, ct, :, :, :])
                        xts[3][ct] = t
                for ot in range(2):
                    ob = ob_pool.tile([P, H, W_SP], f32, name="ob")
                    last = (smp == S - 1)
                    for hb in range(8):
                        pb = ps.tile([P, 8, W_SP], f32, name="pb", bufs=2)
                        first = True
                        for ct in range(2):
                            for d in range(9):
                                di, dj = divmod(d, 3)
                                loc = hb * 8 + di
                                nc.tensor.matmul(
                                    pb[:, :, :],
                                    ws2[ct][:, d, smp, ot * P:(ot + 1) * P],
                                    xts[smp][ct][:, loc:loc + 8, dj:dj + W_SP],
                                    start=first, stop=(ct == 1 and d == 8))
                                first = False
                        if hb % 2 == 0:
                            nc.vector.tensor_copy(
                                ob[:, hb * 8:(hb + 1) * 8, :], pb[:, :, :])
                        else:
                            nc.scalar.copy(
                                ob[:, hb * 8:(hb + 1) * 8, :], pb[:, :, :])
                        if last and ot == 1 and hb >= 4:
                            nc.sync.dma_start(
                                outd[smp, ot, :, hb * 8:(hb + 1) * 8, :],
                                ob[:, hb * 8:(hb + 1) * 8, :])
                        elif last and hb % 2 == 1:
                            k = hb // 2
                            nc.sync.dma_start(
                                outd[smp, ot, :, k * 16:(k + 1) * 16, :],
                                ob[:, k * 16:(k + 1) * 16, :])
                    if not last:
                        nc.sync.dma_start(outd[smp, ot, :, :, :], ob[:, :, :])
    nc.finalize()
    return nc


def _host_prep(x, alphas, W, lora_A, lora_B):
    """Host-side layout/dtype transforms (pad/transpose/gather/cast)."""
    import ml_dtypes
    bf16 = ml_dtypes.bfloat16

    xf = np.asarray(x, dtype=np.float32)
    af = np.asarray(alphas, dtype=np.float32)
    Wf = np.asarray(W, dtype=np.float32)
    Af = np.asarray(lora_A, dtype=np.float32).reshape(NR, CIN * K)   # Acat
    Bf = np.asarray(lora_B, dtype=np.float32)

    # padded x, per core: (S, 2, 128, 66, 66) bf16
    xpad = np.zeros((B, CIN, HP, HP), bf16)
    xpad[:, :, 1:-1, 1:-1] = xf.astype(bf16)
    xpad = xpad.reshape(NCORES, S, 2, P, HP, HP)

    # base weights c-major, d-major free layout, doubled along j so the
    # device-side eviction add needs no broadcast: wt[p, d, ct, j, o]
    wth = np.ascontiguousarray(
        Wf.reshape(COUT, CIN, 9).transpose(1, 2, 0)        # [c, d, o]
        .reshape(2, P, 9, COUT)                            # [ct, p, d, o]
        .transpose(1, 2, 0, 3)).astype(bf16)               # [p, d, ct, o]
    wth = np.ascontiguousarray(
        np.broadcast_to(wth[:, :, :, None, :], (P, 9, 2, 2, COUT)))

    # a3[r, d, s, c] = Acat[r, c*9+d-768*s] masked; rows padded 96 -> 128
    a3h = np.zeros((P, 9, 3, CIN), np.float32)
    cc = np.arange(CIN)
    for d in range(9):
        q = cc * 9 + d
        s_of_c = q // (CIN * K)
        q_of_c = q % (CIN * K)
        for s in range(3):
            m = s_of_c == s
            a3h[:NR, d, s, m] = Af[:, q_of_c[m]]
    a3h = a3h.astype(bf16)

    # b3[r, s, o] = Bcat[3o+s, r];  Bcat = lora_B transposed to [768, 96]
    Bcat = Bf.transpose(1, 0, 2).reshape(COUT * K, NR)
    b3h = np.zeros((P, 3, COUT), np.float32)
    b3h[:NR] = Bcat.reshape(COUT, 3, NR).transpose(2, 1, 0)
    b3h = b3h.astype(bf16)

    # alph[r, smp] per core (repeat each task 24x; zero rows >= 96).
    # SCALING (alpha/r = 2) folded in here so the device skips one mul.
    alphh = np.zeros((NCORES, P, S), np.float32)
    rep = np.repeat(af * SCALING, R * K, axis=1)           # [B, 96]
    alphh[:, :NR, :] = rep.reshape(NCORES, S, NR).transpose(0, 2, 1)

    return xpad, wth, a3h, b3h, alphh


def kernel(x, alphas, W, lora_A, lora_B):
    from concourse.bass_utils import run_bass_kernel_spmd

    if "nc" not in _CACHE:
        _CACHE["nc"] = _build_nc()
    nc = _CACHE["nc"]

    xpad, wth, a3h, b3h, alphh = _host_prep(x, alphas, W, lora_A, lora_B)
    in_maps = [
        {"xp": np.ascontiguousarray(xpad[c]), "wt": wth, "a3": a3h, "b3": b3h,
         "alph": np.ascontiguousarray(alphh[c])}
        for c in range(NCORES)
    ]
    res = run_bass_kernel_spmd(nc, in_maps, list(range(NCORES)))
    out = np.empty((B, COUT, H, W_SP), np.float32)
    for c in range(NCORES):
        out[c * S:(c + 1) * S] = res.results[c]["out"].reshape(S, COUT, H, W_SP)
    return out


# revision 29
# speedup vs baseline: 1.2040x; 1.0002x over previous
# MultiLoraConv2d kernel for 8 trn2 NeuronCores (Bass/Tile, data-parallel over batch).
#
# Math (per sample b):
#   delta_flat[b] = sum_t 2*alphas[b,t] * (lora_B[t] @ lora_A[t])        [768, 768]
#   agg[b] = W + delta_flat[b].reshape(COUT, CIN, 3, 3)                  (flat reinterpret)
#   out[b] = conv2d(x[b], agg[b], pad=1)
#
# Device strategy (per core, S = B/8 samples):
#   - All matmul operands bf16 (1 cycle/row on PE, same as fp32r; halves DMA
#     + SBUF). PSUM accumulation stays fp32; max rel err ~2.3e-3 (gate 2e-2).
#   - Host pre-lays-out tensors partition-first; LoRA factors regrouped by
#     (d = 3*i + j, s = (c*9+d)//768) so per-sample aggregated conv weights
#     come out of the PE directly in c-major (stationary) layout:
#       S_d[c, o] = sum_s sum_r a3[d,s,r,c] * (2*alpha_{t(r)} * b3[s,r,o])
#   - Conv = 18 shifted matmuls (9 taps x 2 cin tiles) per PSUM bank;
#     measured issue cadence ~218 ns per 512-row bf16 matmul (hw floor).
#   - Schedule: 11 dummy warmup matmuls ramp the PE p-state during the ~8us
#     NEFF boot + ~11us first-DMA latency window; DMAs issued in consumption
#     order (alph/b3[s0] first, a3/wt interleaved per-d, then x prefetches);
#     b3s muls split DVE/Act in consumption order; phase-1 evictions on DVE
#     (GpSimd cannot touch PSUM and is ~10x slower on elementwise); conv
#     PSUM->SBUF copies alternate DVE/Act; single 8-bank PSUM pool (6 dp +
#     2 pb ring slots, warmup folded into the dp tag); output batched 1 DMA
#     per (smp, ot), final group split fine to shrink the drain tail.
import numpy as np

B, T, R, ALPHA = 32, 4, 8, 16
CIN, COUT, K = 256, 256, 3
H = W_SP = 64
SCALING = ALPHA / R
NCORES = 8
S = B // NCORES      # samples per core
NR = T * R * K       # 96 lora rows (padded to 128 partitions)
P = 128
HP = H + 2           # 66 padded

_CACHE = {}


def _build_nc():
    import concourse.bacc as bacc
    import concourse.mybir as mybir
    import concourse.tile as tile

    f32 = mybir.dt.float32
    bf16 = mybir.dt.bfloat16

    nc = bacc.Bacc("TRN2", target_bir_lowering=False, debug=False, num_devices=NCORES)

    xp = nc.declare_dram_parameter("xp", [S, 2, P, HP, HP], bf16, isOutput=False)
    wt = nc.declare_dram_parameter("wt", [P, 9, 2, 2, COUT], bf16, isOutput=False)
    a3 = nc.declare_dram_parameter("a3", [NR, 9, 3, CIN], bf16, isOutput=False)
    b3 = nc.declare_dram_parameter("b3", [NR, 3, COUT], bf16, isOutput=False)
    alph = nc.declare_dram_parameter("alph", [NR, S], f32, isOutput=False)
    outd = nc.declare_dram_parameter("out", [S, 2, P, H, W_SP], f32, isOutput=True)

    with tile.TileContext(nc) as tc:
        with tc.tile_pool(name="persist", bufs=1) as persist, \
             tc.tile_pool(name="xt_pool", bufs=6) as xt_pool, \
             tc.tile_pool(name="ob_pool", bufs=2) as ob_pool, \
             tc.tile_pool(name="ps", bufs=1, space="PSUM") as ps:
            a3_sb = persist.tile([P, 9, 3, CIN], bf16)
            b3_sb = persist.tile([P, 3, COUT], bf16)
            alph_sb = persist.tile([P, S], f32)
            wt_sb = persist.tile([P, 9, 2, 2, COUT], bf16)
            b3s = [persist.tile([P, 3, 2, COUT], bf16, name=f"b3s{h}")
                   for h in range(2)]
            ws2 = [persist.tile([P, 9, S, COUT], bf16, name=f"ws2{c}")
                   for c in range(2)]
            wz = persist.tile([P, 704], bf16)

            # PE warmup: p-state ramps to full clock after ~3us of continuous
            # execution; burn that in during NEFF boot / first DMAs so real
            # matmuls run near 2.4GHz from the start. ~11x512-row at mid clock
            # ends ~11.1us, right when the first dp group's operands land
            # (alph DMA completes ~10.9 + two muls). memset on DVE (GpSimd's
            # boot is slow/variable); a dummy Act copy pulls the 1.3us
            # ACT_TABLE_LOAD off the b3s-mul critical path (its target range
            # is outside the warmup APs so it gates nothing).
            nc.vector.memset(wz[:, :], 0.0)
            nc.scalar.copy(wz[:, 672:688], wz[:, 640:656])
            warm = ps.tile([P, 2, COUT], f32, name="dp", bufs=6)
            for _ in range(9):
                nc.tensor.matmul(warm[:, :, :], wz[:, 0:128], wz[:, 128:640],
                                 start=True, stop=True)

            # DMAs in consumption order on the SP queue (issuing from a
            # second DGE queue halves per-queue DMA bandwidth kernel-wide —
            # measured +60us). a3 chunks (stationary, gate matmuls directly)
            # lead their wt peers (which only gate the ring-buffered eviction
            # adds); alph (first transfer, ~10.9us completion due to DMA pipe
            # latency) gates the b3s muls; x prefetches trail (needed ~39us).
            nc.sync.dma_start(alph_sb[0:NR, :], alph[:, :])
            nc.sync.dma_start(b3_sb[0:NR, 0:1], b3[:, 0:1])
            nc.sync.dma_start(b3_sb[0:NR, 1:3], b3[:, 1:3])
            nc.sync.dma_start(a3_sb[0:NR, 0:1], a3[:, 0:1])
            nc.sync.dma_start(a3_sb[0:NR, 1:3], a3[:, 1:3])
            nc.sync.dma_start(wt_sb[:, 0:1], wt[:, 0:1])
            nc.sync.dma_start(a3_sb[0:NR, 3:6], a3[:, 3:6])
            nc.sync.dma_start(wt_sb[:, 1:3], wt[:, 1:3])
            nc.sync.dma_start(a3_sb[0:NR, 6:9], a3[:, 6:9])
            nc.sync.dma_start(wt_sb[:, 3:6], wt[:, 3:6])
            nc.sync.dma_start(wt_sb[:, 6:9], wt[:, 6:9])
            # (wt is host-doubled along a j=2 dim so phase-1 eviction is a
            # single [P,2,256] tensor_add per (h,d,ct) with no broadcast AP)
            xts = [[None, None] for _ in range(S)]
            for smp in range(3):
                for ct in range(2):
                    t = xt_pool.tile([P, HP, HP], bf16, name="xt")
                    nc.sync.dma_start(t[:, :, :], xp[smp, ct, :, :, :])
                    xts[smp][ct] = t

            # b3s[h][:, s, j, :] = (2*alphas[2h+j]) * b3[s]  (bf16 out; the
            # 2x scaling is folded into the host-side alph values).
            # DVE and Act interleave so the first dp group's s=0,1,2 operands
            # land in consumption order with no single-engine serial chain;
            # DVE then goes straight into the h=0 eviction adds.
            for s, j, eng in ((0, 0, nc.vector), (0, 1, nc.vector),
                              (2, 0, nc.vector), (2, 1, nc.vector)):
                eng.tensor_scalar_mul(b3s[0][0:NR, s, j, :], b3_sb[0:NR, s, :],
                                      alph_sb[0:NR, j:j + 1])
            for h, s, j in ((0, 1, 0), (0, 1, 1), (1, 0, 0), (1, 0, 1),
                            (1, 1, 0), (1, 1, 1), (1, 2, 0), (1, 2, 1)):
                nc.scalar.mul(b3s[h][0:NR, s, j, :], b3_sb[0:NR, s, :],
                              alph_sb[0:NR, 2 * h + j:2 * h + j + 1])

            # ---- phase 1: aggregated weights via LoRA matmuls ----
            # dp[c, j, o] = sum_s sum_r a3[d,s,r,c] * b3s[h][s,j,o]; then
            # ws2[ct][c, d, 2h+j, o] = dp[c, j, o] + W[c, d, ct, o].
            for h in range(2):
                for d in range(9):
                    for ct in range(2):
                        dp = ps.tile([P, 2, COUT], f32, name="dp", bufs=6)
                        for s in range(3):
                            nc.tensor.matmul(
                                dp[:, :, :],
                                a3_sb[0:NR, d, s, ct * P:(ct + 1) * P],
                                b3s[h][0:NR, s, :, :],
                                start=(s == 0), stop=(s == 2))
                        nc.vector.tensor_add(
                            ws2[ct][:, d, 2 * h:2 * h + 2, :],
                            dp[:, :, :], wt_sb[:, d, ct, :, :])

            # ---- phase 2: per-sample conv, 18 shifted matmuls per psum bank ----
            for smp in range(S):
                if smp == 1:
                    for ct in range(2):
                        t = xt_pool.tile([P, HP, HP], bf16, name="xt")
                        nc.sync.dma_start(t[:, :, :], xp[3, ct, :, :, :])
                        xts[3][ct] = t
                for ot in range(2):
                    ob = ob_pool.tile([P, H, W_SP], f32, name="ob")
                    last = (smp == S - 1)
                    for hb in range(8):
                        pb = ps.tile([P, 8, W_SP], f32, name="pb", bufs=2)
                        first = True
                        for ct in range(2):
                            for d in range(9):
                                di, dj = divmod(d, 3)
                                loc = hb * 8 + di
                                nc.tensor.matmul(
                                    pb[:, :, :],
                                    ws2[ct][:, d, smp, ot * P:(ot + 1) * P],
                                    xts[smp][ct][:, loc:loc + 8, dj:dj + W_SP],
                                    start=first, stop=(ct == 1 and d == 8))
                                first = False
                        if hb % 2 == 0:
                            nc.vector.tensor_copy(
                                ob[:, hb * 8:(hb + 1) * 8, :], pb[:, :, :])
                        else:
                            nc.scalar.copy(
                                ob[:, hb * 8:(hb + 1) * 8, :], pb[:, :, :])
                        if last and ot == 1 and hb >= 4:
                            nc.sync.dma_start(
                                outd[smp, ot, :, hb * 8:(hb + 1) * 8, :],
                                ob[:, hb * 8:(hb + 1) * 8, :])
                        elif last and hb % 2 == 1:
                            k = hb // 2
                            nc.sync.dma_start(
                                outd[smp, ot, :, k * 16:(k + 1) * 16, :],
                                ob[:, k * 16:(k + 1) * 16, :])
                    if not last:
                        nc.sync.dma_start(outd[smp, ot, :, :, :], ob[:, :, :])
    nc.finalize()
    return nc


def _host_prep(x, alphas, W, lora_A, lora_B):
    """Host-side layout/dtype transforms (pad/transpose/gather/cast)."""
    import ml_dtypes
    bf16 = ml_dtypes.bfloat16

    xf = np.asarray(x, dtype=np.float32)
    af = np.asarray(alphas, dtype=np.float32)
    Wf = np.asarray(W, dtype=np.float32)
    Af = np.asarray(lora_A, dtype=np.float32).reshape(NR, CIN * K)   # Acat
    Bf = np.asarray(lora_B, dtype=np.float32)

    # padded x, per core: (S, 2, 128, 66, 66) bf16
    xpad = np.zeros((B, CIN, HP, HP), bf16)
    xpad[:, :, 1:-1, 1:-1] = xf.astype(bf16)
    xpad = xpad.reshape(NCORES, S, 2, P, HP, HP)

    # base weights c-major, d-major free layout, doubled along j so the
    # device-side eviction add needs no broadcast: wt[p, d, ct, j, o]
    wth = np.ascontiguousarray(
        Wf.reshape(COUT, CIN, 9).transpose(1, 2, 0)        # [c, d, o]
        .reshape(2, P, 9, COUT)                            # [ct, p, d, o]
        .transpose(1, 2, 0, 3)).astype(bf16)               # [p, d, ct, o]
    wth = np.ascontiguousarray(
        np.broadcast_to(wth[:, :, :, None, :], (P, 9, 2, 2, COUT)))

    # a3[r, d, s, c] = Acat[r, c*9+d-768*s] masked; rows padded 96 -> 128
    a3h = np.zeros((P, 9, 3, CIN), np.float32)
    cc = np.arange(CIN)
    for d in range(9):
        q = cc * 9 + d
        s_of_c = q // (CIN * K)
        q_of_c = q % (CIN * K)
        for s in range(3):
            m = s_of_c == s
            a3h[:NR, d, s, m] = Af[:, q_of_c[m]]
    a3h = a3h[:NR].astype(bf16)

    # b3[r, s, o] = Bcat[3o+s, r];  Bcat = lora_B transposed to [768, 96]
    Bcat = Bf.transpose(1, 0, 2).reshape(COUT * K, NR)
    b3h = np.zeros((P, 3, COUT), np.float32)
    b3h[:NR] = Bcat.reshape(COUT, 3, NR).transpose(2, 1, 0)
    b3h = b3h[:NR].astype(bf16)

    # alph[r, smp] per core (repeat each task 24x; zero rows >= 96).
    # SCALING (alpha/r = 2) folded in here so the device skips one mul.
    alphh = np.zeros((NCORES, NR, S), np.float32)
    rep = np.repeat(af * SCALING, R * K, axis=1)           # [B, 96]
    alphh[:, :, :] = rep.reshape(NCORES, S, NR).transpose(0, 2, 1)

    return xpad, wth, a3h, b3h, alphh


def kernel(x, alphas, W, lora_A, lora_B):
    from concourse.bass_utils import run_bass_kernel_spmd

    if "nc" not in _CACHE:
        _CACHE["nc"] = _build_nc()
    nc = _CACHE["nc"]

    xpad, wth, a3h, b3h, alphh = _host_prep(x, alphas, W, lora_A, lora_B)
    in_maps = [
        {"xp": np.ascontiguousarray(xpad[c]), "wt": wth, "a3": a3h, "b3": b3h,
         "alph": np.ascontiguousarray(alphh[c])}
        for c in range(NCORES)
    ]
    res = run_bass_kernel_spmd(nc, in_maps, list(range(NCORES)))
    out = np.empty((B, COUT, H, W_SP), np.float32)
    for c in range(NCORES):
        out[c * S:(c + 1) * S] = res.results[c]["out"].reshape(S, COUT, H, W_SP)
    return out
